# revision 6
# baseline (speedup 1.0000x reference)
"""Trainium2 Bass kernel v2 for nn_DoublyStochasticButterfly.

Feature-major 128-tiles (tile m = feats 128m..128(m+1)-1). Stage t mixes
bit (9-t)%10. Decomposition (folds chosen to balance PE vs DVE vs Pool):

    t=0   (b9): cross pairs (m, m+4)      -> elementwise (DVE/Pool)
    t=1   (b8): pair (0,2) elementwise; pairs (1,3),(4,6),(5,7) folded
    t=2-9 (b7..b0): composed into G1 blocks (PE matmul)
    t=10  (b9): cross pairs (m, m+4)      -> elementwise
    t=11  (b8): pairs (0,2),(1,3) elementwise; (4,6),(5,7) folded
    t=12-19: composed into G2 blocks (PE, swapped operands -> batch-major)

On-chip fp16 (PSUM f32). Input lands fp16 via gpsimd casting DMA, is
transposed by the DMA xbar (dma_start_transpose, SP). Output DMA on
gpsimd (SWDGE does not occupy the Pool engine during transfer).

Sharding: batch dim split across the 8 cores (data parallel, no comm).
"""

import numpy as np

# ---------------------------------------------------------------- constants
WIDTH = 1024
HALF = 512
DEPTH = 20
BATCH = 32768
NCORES = 8
BSH = BATCH // NCORES
CHUNK = 512
NCHUNK = BSH // CHUNK

REPEAT = 1

# executed elementwise pairs per stage: (m0, m1, engine)
# engine: 'dve' = 2 LERP passes; 'pool' = DVE d-pass (fp16 2x TT) + 3 Pool
# TTs using a materialized broadcast weight tile (Pool has no TensorScalarPtr)
CROSS_A = [
    (0, [(0, 4, "dve"), (1, 5, "dve"), (2, 6, "dve"), (3, 7, "pool")]),
]
CROSS_B = [
    (10, [(0, 4, "dve"), (1, 5, "dve"), (2, 6, "dve"), (3, 7, "pool")]),
    (11, [(0, 2, "dve"), (1, 3, "dve")]),
]
T1_FOLDS = [(0, 2), (1, 3), (4, 6), (5, 7)]  # stage-1 pairs folded into G1
T11_FOLDS = [(4, 6), (5, 7)]  # stage-11 pairs folded into G2
POOL_BCAST = [(0, 3, 7), (10, 3, 7)]  # pairs needing broadcast weight tiles


def _rotr(i, t):
    for _ in range(t):
        i = (i >> 1) | ((i & 1) << 9)
    return i


def _stage_pairs(t):
    b = (9 - t) % 10
    i0 = np.array([_rotr(p, t) for p in range(HALF)])
    i1 = i0 | (1 << b)
    return i0, i1


def _stage_matrix(t, p64, only_pairs=None):
    """Stage matrix; only_pairs restricts to tile-pairs in the list
    (identity elsewhere)."""
    m = np.eye(WIDTH)
    i0, i1 = _stage_pairs(t)
    w = p64[:, t].copy()
    if only_pairs is not None:
        keep = np.zeros(HALF, dtype=bool)
        for m0, m1 in only_pairs:
            keep |= (i0 // 128 == m0) & (i1 // 128 == m1)
        i0, i1, w = i0[keep], i1[keep], w[keep]
    m[i0, i0] = 1 - w
    m[i0, i1] = w
    m[i1, i0] = w
    m[i1, i1] = 1 - w
    return m


def _pair_weights(t, p64):
    """Per-pair per-partition weight vectors: {(m0,m1): w[128]}."""
    i0, i1 = _stage_pairs(t)
    wt = np.zeros(WIDTH)
    wt[i0] = p64[:, t]
    out = {}
    for m0 in range(8):
        for m1 in range(m0 + 1, 8):
            sel = (i0 // 128 == m0) & (i1 // 128 == m1)
            if sel.any():
                out[(m0, m1)] = wt[128 * m0 : 128 * (m0 + 1)]
    return out


def _host_precompute(params):
    p64 = np.asarray(params, dtype=np.float64)

    def composed(ts):
        g = np.eye(WIDTH)
        for t in ts:
            g = _stage_matrix(t, p64) @ g
        return g

    # G1 = M9..M2 . M1^{T1_FOLDS};  G2 = M19..M12 . M11^{T11_FOLDS}
    g1 = np.eye(WIDTH)
    g1 = _stage_matrix(1, p64, only_pairs=T1_FOLDS) @ g1
    for t in range(2, 10):
        g1 = _stage_matrix(t, p64) @ g1
    g2 = np.eye(WIDTH)
    g2 = _stage_matrix(11, p64, only_pairs=T11_FOLDS) @ g2
    for t in range(12, 20):
        g2 = _stage_matrix(t, p64) @ g2

    def blocks_nonzero(g, out_rows):
        """j-list of nonzero 128-col blocks for a row range."""
        return [
            j
            for j in range(8)
            if np.abs(g[out_rows, 128 * j : 128 * (j + 1)]).max() > 1e-15
        ]

    # G1 lhsT packing: for out-tile k, j-list; lhsT block = g1[kblk, jblk].T
    wl_off = {}
    wl_cols = []
    for k in range(8):
        rows = slice(128 * k, 128 * (k + 1))
        for j in blocks_nonzero(g1, rows):
            wl_off[(k, j)] = 128 * len(wl_cols)
            wl_cols.append(g1[rows, 128 * j : 128 * (j + 1)].T)
    wl_pack = np.concatenate(wl_cols, axis=1)

    # G2 rhs packing (swapped operands): out 256-block q, rhs = g2_blk.T
    wr_off = {}
    wr_cols = []
    for q in range(4):
        rows = slice(256 * q, 256 * (q + 1))
        for j in blocks_nonzero(g2, rows):
            wr_off[(q, j)] = 256 * len(wr_cols)
            wr_cols.append(g2[rows, 128 * j : 128 * (j + 1)].T)
    wr_pack = np.concatenate(wr_cols, axis=1)

    # cross weights: per executed pair, columns (+w, -w)
    wc_cols = []
    wc_off = {}
    for stages in (CROSS_A, CROSS_B):
        for t, pairs in stages:
            pw = _pair_weights(t, p64)
            for m0, m1, eng in pairs:
                w = pw[(m0, m1)]
                wc_off[(t, m0, m1)] = len(wc_cols)
                wc_cols.append(w)
                wc_cols.append(-w)
    wc_pack = np.stack(wc_cols, axis=1)

    # ---- end-to-end verification (f64) ----
    g_total = composed(range(DEPTH))

    def lerp(x0, x1, w):
        return (x1 - x0) * w[:, None] + x0

    cur = [np.eye(WIDTH)[128 * m : 128 * (m + 1)] for m in range(8)]
    for t, pairs in CROSS_A:
        pw = _pair_weights(t, p64)
        for m0, m1, eng in pairs:
            w = pw[(m0, m1)]
            a = lerp(cur[m0], cur[m1], w)
            b = lerp(cur[m1], cur[m0], w)
            cur[m0], cur[m1] = a, b
    nxt = []
    for k in range(8):
        acc = np.zeros((128, WIDTH))
        for j in range(8):
            if (k, j) in wl_off:
                o = wl_off[(k, j)]
                acc += wl_pack[:, o : o + 128].T @ cur[j]
        nxt.append(acc)
    cur = nxt
    for t, pairs in CROSS_B:
        pw = _pair_weights(t, p64)
        for m0, m1, eng in pairs:
            w = pw[(m0, m1)]
            a = lerp(cur[m0], cur[m1], w)
            b = lerp(cur[m1], cur[m0], w)
            cur[m0], cur[m1] = a, b
    y = np.zeros((WIDTH, WIDTH))
    for q in range(4):
        acc = np.zeros((256, WIDTH))
        for j in range(8):
            if (q, j) in wr_off:
                o = wr_off[(q, j)]
                acc += wr_pack[:, o : o + 256].T @ cur[j]
        y[256 * q : 256 * (q + 1)] = acc
    err = np.abs(y - g_total).max()
    assert err < 1e-9, f"decomposition mismatch: {err}"

    return (
        wc_pack.astype(np.float32),
        wl_pack.astype(np.float16),
        wr_pack.astype(np.float16),
        wl_off,
        wr_off,
        wc_off,
    )


_SHAPES = None


def _pack_shapes(params):
    """Column counts depend only on the fold config — compute once."""
    global _SHAPES
    if _SHAPES is None:
        wc, wl, wr, wl_off, wr_off, wc_off = _host_precompute(
            np.asarray(params, dtype=np.float32)
        )
        _SHAPES = (wc.shape[1], wl.shape[1], wr.shape[1], wl_off, wr_off, wc_off)
    return _SHAPES


# ---------------------------------------------------------------- custom op
_LERP = None


def _register_lerp():
    """out = (in0 - in1)*s0 + in1, s0 per-partition."""
    global _LERP
    if _LERP is not None:
        return _LERP
    from concourse import dve_ops as D
    from concourse.dve_spec import C0, Spec, Src0, Src1, lower
    from concourse.dve_uop import DveOpSpec

    name = "LERP_ANT_BFLY"
    for op in D.OPS:
        if op.name == name:
            _LERP = op
            return op

    def _ref(in0, in1, s0, s1, imm2):
        s = np.asarray(s0).reshape(np.asarray(s0).shape[0], *([1] * (in0.ndim - 1)))
        return (in0 - in1) * s + in1

    spec = Spec(body=(Src0 - Src1) * C0 + Src1, reference=_ref)
    opcode = D._CUSTOM_DVE_ROW_BASE + len(D.OPS)
    shas = {}
    for ver in ("v3", "v4"):
        uops = lower(spec, ver=ver)
        shas[ver] = DveOpSpec(name=name, opcode=opcode, uops=uops, rd1_en=True).sha(
            ver
        )
    op = D.DveOp(name, spec, subdim=False, uops_sha=shas)
    D.OPS.append(op)
    D.CUSTOM_DVE_SPECS[name] = spec
    D._SUB_OPCODE_FOR_NAME[name] = opcode
    _LERP = op
    return op


# ---------------------------------------------------------------- bass build
_NC_CACHE = {}


def _build_nc(repeat=REPEAT, shapes=None):
    key = repeat
    if key in _NC_CACHE:
        return _NC_CACHE[key]
    if shapes is None:
        shapes = _pack_shapes(np.random.default_rng(1).random((HALF, DEPTH)))
    ncw, nwl, nwr, wl_off, wr_off, wc_off = shapes
    REP = repeat
    import contextlib

    import concourse.mybir as mybir
    import concourse.tile as tile
    from concourse import bacc

    lerp = _register_lerp()
    f32 = mybir.dt.float32
    f16 = mybir.dt.float16
    AO = mybir.AluOpType

    nc = bacc.Bacc("TRN2", target_bir_lowering=False, debug=False,
                   num_devices=NCORES)
    x_d = nc.dram_tensor("X", [BSH, WIDTH], f32, kind="ExternalInput").ap()
    wl_d = nc.dram_tensor("WL", [128, nwl], f16, kind="ExternalInput").ap()
    wr_d = nc.dram_tensor("WR", [128, nwr], f16, kind="ExternalInput").ap()
    wc_d = nc.dram_tensor("WC", [128, ncw], f32, kind="ExternalInput").ap()
    if POOL_BCAST:
        wb_d = nc.dram_tensor(
            "WB", [128, 512 * len(POOL_BCAST)], f16, kind="ExternalInput"
        ).ap()
    y_d = nc.dram_tensor("Y", [BSH, WIDTH], f32, kind="ExternalOutput").ap()

    with tile.TileContext(nc) as tc:
        with (
            tc.tile_pool(name="wts", bufs=1) as wpool,
            tc.tile_pool(name="io", bufs=2) as iop,
            tc.tile_pool(name="work", bufs=2) as wk,
            tc.tile_pool(name="psb", bufs=2, space="PSUM") as psB,
            tc.tile_pool(name="psc", bufs=2, space="PSUM") as psC,
        ):
            wl = wpool.tile([128, nwl], f16, tag="wl")
            wr = wpool.tile([128, nwr], f16, tag="wr")
            wc = wpool.tile([128, ncw], f32, tag="wc")
            nc.sync.dma_start(out=wl[:], in_=wl_d[:])
            nc.sync.dma_start(out=wr[:], in_=wr_d[:])
            nc.sync.dma_start(out=wc[:], in_=wc_d[:])
            # host-precomputed broadcast weight tiles for pool pairs
            wb_tiles = {}
            if POOL_BCAST:
                wball = wpool.tile([128, 512 * len(POOL_BCAST)], f16, tag="wb")
                nc.sync.dma_start(out=wball[:], in_=wb_d[:])
                for i, (t, m0, m1) in enumerate(POOL_BCAST):
                    wb_tiles[(t, m0, m1)] = wball[:, 512 * i : 512 * (i + 1)]

            xh_of, xt_of, ca_of = {}, {}, {}
            ub_of, vb_of, yo_of = {}, {}, {}
            psb_of, psc_of = {}, {}

            def cross_pair(tag, c, t, m0, m1, eng, x0, x1, a, b):
                o = wc_off[(t, m0, m1)]
                w = wc[:, o : o + 1]
                if eng == "dve":
                    nc.vector._custom_dve(lerp, out=a, in0=x1, in1=x0, s0=w)
                    nc.vector._custom_dve(lerp, out=b, in0=x0, in1=x1, s0=w)
                else:  # pool: d on DVE (fp16 2x TT), 3 TTs on Pool
                    d = wk.tile([128, 512], f16, tag=f"d{tag}_{t}_{m0}",
                                name=f"d{tag}{c}_{t}_{m0}")
                    m = wk.tile([128, 512], f16, tag=f"m{tag}_{t}_{m0}",
                                name=f"m{tag}{c}_{t}_{m0}")
                    wb = wb_tiles[(t, m0, m1)]
                    nc.vector.tensor_tensor(d[:], x1, x0, AO.subtract)
                    nc.gpsimd.tensor_tensor(m[:], d[:], wb, AO.mult)
                    nc.gpsimd.tensor_tensor(a, x0, m[:], AO.add)
                    nc.gpsimd.tensor_tensor(b, x1, m[:], AO.subtract)

            def run_cross(tag, c, stages, cur):
                """cur: list of 8 (ap, contiguous_tile_or_None). Applies
                stages, allocating output tiles; returns new cur aps."""
                for t, pairs in stages:
                    for m0, m1, eng in pairs:
                        a = wk.tile([128, 512], f16, tag=f"x{tag}_{t}_{m0}",
                                    name=f"x{tag}{c}_{t}_{m0}")
                        b = wk.tile([128, 512], f16, tag=f"x{tag}_{t}_{m1}",
                                    name=f"x{tag}{c}_{t}_{m1}")
                        cross_pair(tag, c, t, m0, m1, eng,
                                   cur[m0], cur[m1], a[:], b[:])
                        cur[m0], cur[m1] = a[:], b[:]
                return cur

            def dma_in(c):
                xh = iop.tile([128, 4096], f16, tag="xh", bufs=2, name=f"xh{c}")
                r0 = c * CHUNK
                nc.gpsimd.dma_start(
                    out=xh[:].rearrange("p (s f) -> p s f", f=WIDTH),
                    in_=x_d[r0 : r0 + CHUNK, :].rearrange("(s p) f -> p s f", p=128),
                )
                xh_of[c] = xh

            def xpose(c):
                xh = xh_of.pop(c)
                xt = iop.tile([128, 4096], f16, tag="xt", bufs=2, name=f"xt{c}")
                nc.sync.dma_start_transpose(
                    xt[:].rearrange("p (k b) -> p k b", k=32), xh[:]
                )
                xt_of[c] = xt

            def crossA(c):
                xt3 = xt_of.pop(c)[:].rearrange("p (s m b) -> p s m b", s=4, m=8)
                cur = [xt3[:, :, m, :] for m in range(8)]
                ca_of[c] = run_cross("a", c, CROSS_A, cur)

            def g1(c, qt):
                cur = ca_of[c]
                psb = psB.tile([128, 1024], f32, tag="psb", name=f"psb{c}_{qt}")
                for k in (0, 4, 1, 5, 2, 6, 3, 7):
                    dst = psb[:, 128 * k : 128 * (k + 1)]
                    js = [j for j in range(8) if (k, j) in wl_off]
                    for ji, j in enumerate(js):
                        o = wl_off[(k, j)]
                        rhs = cur[j]
                        # cur[j] may be a 3D strided xt view or [128,512] tile
                        if len(rhs.shape) == 3:
                            rhs = rhs[:, qt, :]
                        else:
                            rhs = rhs[:, 128 * qt : 128 * (qt + 1)]
                        nc.tensor.matmul(
                            dst,
                            wl[:, o : o + 128],
                            rhs,
                            start=(ji == 0),
                            stop=(ji == len(js) - 1),
                        )
                psb_of[(c, qt)] = psb
                if qt == 3:
                    ca_of.pop(c)

            def evac1(c, qt):
                psb = psb_of.pop((c, qt))
                if qt == 0:
                    ub_of[c] = wk.tile([128, 8 * 512], f16, tag="ub", name=f"ub{c}")
                ub = ub_of[c]
                nc.scalar.copy(
                    ub[:].rearrange("p (m b) -> p m b", m=8)[
                        :, :, 128 * qt : 128 * (qt + 1)
                    ],
                    psb[:].rearrange("p (m b) -> p m b", m=8),
                )

            def crossB(c):
                ub = ub_of.pop(c)
                cur = [ub[:, 512 * m : 512 * (m + 1)] for m in range(8)]
                vb_of[c] = run_cross("b", c, CROSS_B, cur)

            def g2(c, s):
                cur = vb_of[c]
                psc = psC.tile([128, 1024], f32, tag="psc", name=f"psc{c}_{s}")
                for q in range(4):
                    dst = psc[:, 256 * q : 256 * (q + 1)]
                    js = [j for j in range(8) if (q, j) in wr_off]
                    for ji, j in enumerate(js):
                        o = wr_off[(q, j)]
                        lhsT = cur[j][:, 128 * s : 128 * (s + 1)]
                        nc.tensor.matmul(
                            dst,
                            lhsT,
                            wr[:, o : o + 256],
                            start=(ji == 0),
                            stop=(ji == len(js) - 1),
                        )
                psc_of[(c, s)] = psc
                if s == 3:
                    vb_of.pop(c)

            def evac2(c, s):
                psc = psc_of.pop((c, s))
                if s == 0:
                    yo_of[c] = iop.tile([128, 4096], f32, tag="yo", name=f"yo{c}")
                yo = yo_of[c]
                nc.scalar.copy(yo[:, 1024 * s : 1024 * (s + 1)], psc[:])

            def dma_out(c):
                # Engines are blocked ~dest-bytes/partition * DMA_CYCLE per
                # DMA. Early chunks alternate Pool/SP; the LAST two chunks go
                # on ACT so the Pool/SP stream tails stay clear of the next
                # REPEAT iteration's input chain (Pool: dma_in, SP: xpose).
                yo = yo_of.pop(c)
                r0 = c * CHUNK
                eng = nc.gpsimd if c % 2 == 0 else nc.sync
                eng.dma_start(
                    out=y_d[r0 : r0 + CHUNK, :].rearrange("(s p) f -> p s f", p=128),
                    in_=yo[:].rearrange("p (s f) -> p s f", f=WIDTH),
                )

            rep_ctx = (
                tc.For_i(0, REP, 1) if REP > 1 else contextlib.nullcontext()
            )
            with rep_ctx:
                for it in range(NCHUNK + 3):
                    if it < NCHUNK:
                        dma_in(it)
                    cA = it - 1
                    if 0 <= cA < NCHUNK:
                        xpose(cA)
                        crossA(cA)
                    cB = it - 2
                    if 0 <= cB < NCHUNK:
                        for qt in range(4):
                            g1(cB, qt)
                            evac1(cB, qt)
                        crossB(cB)
                    cC = it - 3
                    if 0 <= cC < NCHUNK:
                        for s in range(4):
                            g2(cC, s)
                            evac2(cC, s)
                        dma_out(cC)

    nc.finalize()
    _NC_CACHE[key] = nc
    return nc


# ---------------------------------------------------------------- entry
def _in_maps(X, params):
    X = np.ascontiguousarray(np.asarray(X, dtype=np.float32))
    wc, wl, wr, wl_off, wr_off, wc_off = _host_precompute(params)
    global _SHAPES
    _SHAPES = (wc.shape[1], wl.shape[1], wr.shape[1], wl_off, wr_off, wc_off)
    wbcols = []
    for t, m0, m1 in POOL_BCAST:
        o = wc_off[(t, m0, m1)]
        wbcols.append(np.repeat(wc[:, o : o + 1], 512, axis=1))
    base = {"WL": wl, "WR": wr, "WC": wc}
    if wbcols:
        base["WB"] = np.concatenate(wbcols, axis=1).astype(np.float16)
    return [
        {"X": X[c * BSH : (c + 1) * BSH], **base} for c in range(NCORES)
    ]


def kernel(X, params):
    in_maps = _in_maps(X, params)
    nc = _build_nc()

    from concourse.bass_utils import run_bass_kernel_spmd

    res = run_bass_kernel_spmd(nc, in_maps, core_ids=list(range(NCORES)))
    return np.concatenate([res.results[c]["Y"] for c in range(NCORES)], axis=0)
